# revision 7
# baseline (speedup 1.0000x reference)
"""Trainium2 Bass kernel for EquivariantMPLayer (GNN message passing).

  msg_repr = [x[row], x[col], edge_dist]            # [E, 2C+1]
  messages = relu(msg_repr @ W_msg + b_msg)         # [E, H]
  aggr     = segment_sum(messages, col, N)          # [N, H]
  out      = x @ W_res + relu([x, aggr] @ W_upd + b_upd)

Strategy (8 NeuronCores, SPMD single program):
  * Host: sort edges by col; shard cores by contiguous node ranges, so each
    core's local segment-sum is the complete aggregate for its node slice --
    no cross-core reduction at all. Within a core, nodes are split into
    variable-width blocks (<=126 nodes, <=T*128 edges) so edge tiles are
    ~95% full; every block gets exactly T tiles (uniform SPMD stream).
  * Per edge tile (128 edges): indirect-DMA gather x[row] (the only
    per-edge random access; SWDGE descriptor-generation bound), then on PE:
      pre[e,H] = x_row @ W1  +  bt^T @ c_aug
    where bt[e, 0:126] is the one-hot col indicator (one DVE is-equal
    against an iota constant), bt[e,126]=valid, bt[e,127]=dist, and
    c_aug = [x_block @ W2 ; b_msg ; w3]. One matmul applies the col-side
    message term, the bias, and the dist*w3 term at once.
      msg = relu(pre)                   (ACT)
      aggr_block += bt[:,0:126]^T @ msg (PE, PSUM accumulation over tiles)
    Stationary matmul operands are bf16 (2x faster LDWEIGHTS).
  * Node update per block is a few 128x128 matmuls against the kept x^T.
"""
import numpy as np
import ml_dtypes
import os
BF16 = bool(int(os.environ.get("K_BF16", "1")))

N = 50000
E = 800000
C = 128
H = 128
NCORES = 8
BLK = 126                    # max nodes per block
TB = 16                      # tiles per block (uniform)
ECAP = TB * 128              # max edges per block
NODES_PER_CORE = 6300        # fixed contiguous node range per core


def _build_and_run(in_maps, NB):
    import concourse.bacc as bacc
    import concourse.tile as tile
    from concourse import bass, mybir
    from concourse.bass_utils import run_bass_kernel_spmd

    f32 = mybir.dt.float32
    bf16 = mybir.dt.bfloat16 if BF16 else mybir.dt.float32
    i32 = mybir.dt.int32
    P = 128
    T = TB

    nc = bacc.Bacc("TRN2")
    nc.cache_partition_id()

    node_embed = nc.dram_tensor("node_embed", [N, C], f32, kind="ExternalInput")
    x_blocks = nc.dram_tensor("x_blocks", [NB, P, C], f32, kind="ExternalInput")
    gidx = nc.dram_tensor("gidx", [NB, P, T], i32, kind="ExternalInput")
    colmod = nc.dram_tensor("colmod", [NB, P, T], f32, kind="ExternalInput")
    cmrows = nc.dram_tensor("cmrows", [NB, T, P], f32, kind="ExternalInput")
    tailrows = nc.dram_tensor("tailrows", [NB, T, 2, P], bf16, kind="ExternalInput")
    iotap_d = nc.dram_tensor("iotap", [P, P], f32, kind="ExternalInput")
    W1 = nc.dram_tensor("W1", [C, H], bf16, kind="ExternalInput")
    W2 = nc.dram_tensor("W2", [C, H], f32, kind="ExternalInput")
    bmsg_w3 = nc.dram_tensor("bmsg_w3", [2, H], bf16, kind="ExternalInput")
    Wu1 = nc.dram_tensor("Wu1", [C, H], f32, kind="ExternalInput")
    Wu2 = nc.dram_tensor("Wu2", [H, H], f32, kind="ExternalInput")
    Wres = nc.dram_tensor("Wres", [C, H], f32, kind="ExternalInput")
    bupd = nc.dram_tensor("bupd", [H, 1], f32, kind="ExternalInput")
    iota = nc.dram_tensor("iota", [P, P], f32, kind="ExternalInput")
    ident = nc.dram_tensor("ident", [P, P], f32, kind="ExternalInput")
    identb = nc.dram_tensor("identb", [P, P], bf16, kind="ExternalInput")
    out_d = nc.dram_tensor("out", [NB * BLK, H], f32, kind="ExternalOutput")

    RELU = mybir.ActivationFunctionType.Relu
    EQ = mybir.AluOpType.is_equal

    with tile.TileContext(nc) as tc:
        with tc.tile_pool(name="const", bufs=1) as cp, \
             tc.tile_pool(name="persist", bufs=1) as pp, \
             tc.tile_pool(name="work", bufs=8) as wp, \
             tc.tile_pool(name="xg", bufs=16) as gp, \
             tc.tile_pool(name="psum3", bufs=3, space="PSUM") as ps, \
             tc.tile_pool(name="psum2", bufs=2, space="PSUM") as ps2:

            def load_const(t, name):
                tl = cp.tile(list(t.shape), t.dtype, tag=name)
                nc.sync.dma_start(out=tl[:], in_=t[:])
                return tl

            w1 = load_const(W1, "w1")
            w2 = load_const(W2, "w2")
            wu1 = load_const(Wu1, "wu1")
            wu2 = load_const(Wu2, "wu2")
            wres = load_const(Wres, "wres")
            bu = load_const(bupd, "bu")
            io_t = load_const(iota, "iota")
            io_p = load_const(iotap_d, "iotap")
            idt = load_const(ident, "ident")
            idtb = load_const(identb, "identb")

            # ---------- phase C: per-block x^T and c_aug ----------
            xT = []
            caug = []
            for b in range(NB):
                xb = wp.tile([P, C], f32, tag="xb")
                nc.sync.dma_start(out=xb[:], in_=x_blocks[b])
                ptx = ps.tile([P, P], f32, space="PSUM", tag="ptx")
                nc.tensor.transpose(out=ptx[:], in_=xb[:], identity=idt[:])
                xt = pp.tile([C, P], f32, tag=f"xT{b}")
                nc.scalar.copy(out=xt[:], in_=ptx[:])
                pc = ps.tile([P, H], f32, space="PSUM", tag="pre")
                nc.tensor.matmul(out=pc[:], lhsT=xt[:], rhs=w2[:], start=True, stop=True)
                ca = pp.tile([P, H], bf16, tag=f"caug{b}")
                nc.vector.tensor_copy(out=ca[0:BLK, :], in_=pc[0:BLK, :])
                nc.sync.dma_start(out=ca[BLK:P, :], in_=bmsg_w3[:])
                xT.append(xt)
                caug.append(ca)

            # ---------- phase E: edges; phase U: node update ----------
            for b in range(NB):
                gix = wp.tile([P, T], i32, tag="gix")
                nc.sync.dma_start(out=gix[:], in_=gidx[b])
                cmod = wp.tile([P, T], f32, tag="cmod")
                nc.sync.dma_start(out=cmod[:], in_=colmod[b])

                pagg = ps2.tile([P, H], f32, space="PSUM", tag="agg")
                for t in range(T):
                    xg = gp.tile([P, C], f32, tag="xg")
                    nc.gpsimd.indirect_dma_start(
                        out=xg[:], out_offset=None, in_=node_embed[:],
                        in_offset=bass.IndirectOffsetOnAxis(ap=gix[:, t:t + 1], axis=0))
                    ptx = ps.tile([P, P], f32, space="PSUM", tag="ptx")
                    nc.tensor.transpose(out=ptx[:], in_=xg[:], identity=idt[:])
                    xts = wp.tile([C, P], bf16, tag="xts")
                    nc.vector.tensor_copy(out=xts[:], in_=ptx[:])

                    bt = gp.tile([P, P], bf16, tag="bt")
                    nc.vector.tensor_scalar(bt[:], io_t[:], cmod[:, t:t + 1], None, EQ)
                    cmrep = gp.tile([P, P], f32, tag="cmrep")
                    nc.sync.dma_start(out=cmrep[:], in_=cmrows[b][t:t + 1, :].to_broadcast([P, P]))
                    btT = gp.tile([P, P], bf16, tag="btT")
                    nc.vector.tensor_tensor(out=btT[:], in0=io_p[:], in1=cmrep[:], op=EQ)
                    nc.sync.dma_start(out=btT[BLK:P, :], in_=tailrows[b][t])

                    ppre = ps.tile([P, H], f32, space="PSUM", tag="pre")
                    nc.tensor.matmul(out=ppre[:], lhsT=xts[:], rhs=w1[:], start=True, stop=False)
                    nc.tensor.matmul(out=ppre[:], lhsT=btT[:], rhs=caug[b][:], start=False, stop=True)
                    msg = wp.tile([P, H], bf16, tag="msg")
                    nc.scalar.activation(out=msg[:], in_=ppre[:], func=RELU)
                    nc.tensor.matmul(out=pagg[0:BLK, :], lhsT=bt[:, 0:BLK], rhs=msg[:],
                                     start=(t == 0), stop=(t == T - 1))

                # ----- node update for block b -----
                aggs = wp.tile([P, H], f32, tag="aggs")
                nc.vector.memset(aggs[:], 0)
                nc.vector.tensor_copy(out=aggs[0:BLK, :], in_=pagg[0:BLK, :])
                pat = ps.tile([P, P], f32, space="PSUM", tag="ptx")
                nc.tensor.transpose(out=pat[:], in_=aggs[:], identity=idt[:])
                aggT = wp.tile([H, P], f32, tag="aggT")
                nc.vector.tensor_copy(out=aggT[:], in_=pat[:])

                pupd = ps.tile([H, P], f32, space="PSUM", tag="pre")
                nc.tensor.matmul(out=pupd[:], lhsT=wu1[:], rhs=xT[b][:], start=True, stop=False)
                nc.tensor.matmul(out=pupd[:], lhsT=wu2[:], rhs=aggT[:], start=False, stop=True)
                rel = wp.tile([H, P], f32, tag="rel")
                nc.scalar.activation(out=rel[:], in_=pupd[:], func=RELU, bias=bu[:])

                pout = ps.tile([H, P], f32, space="PSUM", tag="ptx")
                nc.tensor.matmul(out=pout[:], lhsT=wres[:], rhs=xT[b][:], start=True, stop=True)
                outT = wp.tile([H, P], f32, tag="outT")
                nc.vector.tensor_tensor(out=outT[:], in0=pout[:], in1=rel[:],
                                        op=mybir.AluOpType.add)
                pfin = ps.tile([P, H], f32, space="PSUM", tag="ptx")
                nc.tensor.transpose(out=pfin[:], in_=outT[:], identity=idt[:])
                outs = wp.tile([P, H], f32, tag="outs")
                nc.scalar.copy(out=outs[:], in_=pfin[:])
                nc.sync.dma_start(out=out_d[b * BLK:(b + 1) * BLK, :], in_=outs[0:BLK, :])

    nc.finalize()
    res = run_bass_kernel_spmd(nc, in_maps, core_ids=list(range(NCORES)),
                               trace=bool(int(__import__("os").environ.get("K_TRACE", "0"))))
    return res


def kernel(node_embed, edge_dist, edge_index, W_res, W_msg, b_msg, W_upd, b_upd):
    node_embed = np.asarray(node_embed, dtype=np.float32)
    edge_dist = np.asarray(edge_dist, dtype=np.float32).reshape(-1)
    row = np.asarray(edge_index[0], dtype=np.int64).astype(np.int32)
    col = np.asarray(edge_index[1], dtype=np.int64).astype(np.int32)
    W_res = np.asarray(W_res, dtype=np.float32)
    W_msg = np.asarray(W_msg, dtype=np.float32)
    b_msg = np.asarray(b_msg, dtype=np.float32)
    W_upd = np.asarray(W_upd, dtype=np.float32)
    b_upd = np.asarray(b_upd, dtype=np.float32)

    order = np.argsort(col, kind="stable")
    scol = col[order]
    srow = row[order]
    sdist = edge_dist[order]

    # per-core greedy blocks: <=BLK nodes, <=ECAP edges
    core_blocks = []   # per core: list of (node_start, node_end, e0, e1)
    for core in range(NCORES):
        n0 = core * NODES_PER_CORE
        n1 = min(n0 + NODES_PER_CORE, N)
        blocks = []
        v = n0
        while v < n1:
            vmax = min(v + BLK, n1)
            e0 = np.searchsorted(scol, v)
            emax = np.searchsorted(scol, vmax)
            if emax - e0 <= ECAP:
                vend = vmax
                e1 = emax
            else:
                # find largest vend with edge count <= ECAP
                e1 = e0 + ECAP
                vend = int(scol[e1 - 1])  # last fully-included node candidate
                # all edges of node vend must fit; back off to node boundary
                e1 = np.searchsorted(scol, vend)
                vend = max(vend, v + 1)
                e1 = np.searchsorted(scol, vend)
            blocks.append((v, vend, int(e0), int(e1)))
            v = vend
        core_blocks.append(blocks)

    NB = max(len(b) for b in core_blocks)
    P = 128
    T = TB
    gidx = np.zeros((NCORES, NB, P, T), np.int32)
    colm = np.full((NCORES, NB, P, T), -1.0, np.float32)
    tailh = np.zeros((NCORES, NB, P, 2 * T), np.float32)
    x_blocks = np.zeros((NCORES, NB, P, C), np.float32)

    for core in range(NCORES):
        for b, (v0, v1, e0, e1) in enumerate(core_blocks[core]):
            cnt = e1 - e0
            if cnt:
                idx = np.arange(cnt)
                tt, pp_ = idx // 128, idx % 128
                gidx[core, b, pp_, tt] = srow[e0:e1]
                colm[core, b, pp_, tt] = (scol[e0:e1] - v0).astype(np.float32)
                tailh[core, b, pp_, 2 * tt] = 1.0
                tailh[core, b, pp_, 2 * tt + 1] = sdist[e0:e1]
            x_blocks[core, b, 0:v1 - v0, :] = node_embed[v0:v1]

    iota = np.tile(np.arange(P, dtype=np.float32), (P, 1))
    iota[:, BLK:] = -5.0
    iotap = np.repeat(np.arange(P, dtype=np.float32)[:, None], P, axis=1)
    iotap[BLK:, :] = -6.0
    cmrows = np.ascontiguousarray(colm.transpose(0, 1, 3, 2))
    tailrows = np.ascontiguousarray(
        tailh.reshape(NCORES, NB, P, T, 2).transpose(0, 1, 3, 4, 2)
    ).astype(ml_dtypes.bfloat16 if BF16 else np.float32)
    consts = {
        "W1": W_msg[0:C].astype(ml_dtypes.bfloat16 if BF16 else np.float32),
        "W2": W_msg[C:2 * C],
        "bmsg_w3": np.stack([b_msg, W_msg[2 * C]]).astype(ml_dtypes.bfloat16 if BF16 else np.float32),
        "Wu1": W_upd[0:C], "Wu2": W_upd[C:C + H],
        "Wres": W_res, "bupd": b_upd.reshape(H, 1),
        "iota": iota, "iotap": iotap, "ident": np.eye(P, dtype=np.float32),
        "identb": np.eye(P).astype(ml_dtypes.bfloat16 if BF16 else np.float32),
    }
    in_maps = []
    for core in range(NCORES):
        m = {"node_embed": node_embed, "x_blocks": x_blocks[core],
             "gidx": gidx[core], "colmod": colm[core],
             "cmrows": cmrows[core], "tailrows": tailrows[core]}
        m.update(consts)
        in_maps.append(m)

    res = _build_and_run(in_maps, NB)
    kernel._last_result = res

    out = np.empty((N, H), np.float32)
    for core in range(NCORES):
        o = res.results[core]["out"]
        for b, (v0, v1, _, _) in enumerate(core_blocks[core]):
            out[v0:v1] = o[b * BLK:b * BLK + (v1 - v0)]
    return out


# revision 8
# speedup vs baseline: 1.4963x; 1.4963x over previous
"""Trainium2 Bass kernel for EquivariantMPLayer (GNN message passing).

  msg_repr = [x[row], x[col], edge_dist]            # [E, 2C+1]
  messages = relu(msg_repr @ W_msg + b_msg)         # [E, H]
  aggr     = segment_sum(messages, col, N)          # [N, H]
  out      = x @ W_res + relu([x, aggr] @ W_upd + b_upd)

Strategy (8 NeuronCores, SPMD single program):
  * Host: sort edges by col; shard cores by contiguous node ranges, so each
    core's local segment-sum is the complete aggregate for its node slice --
    no cross-core reduction at all. Within a core, nodes are split into
    variable-width blocks (<=126 nodes, <=T*128 edges) so edge tiles are
    ~95% full; every block gets exactly T tiles (uniform SPMD stream).
  * Per edge tile (128 edges): indirect-DMA gather x[row] (the only
    per-edge random access; SWDGE descriptor-generation bound), then on PE:
      pre[e,H] = x_row @ W1  +  bt^T @ c_aug
    where bt[e, 0:126] is the one-hot col indicator (one DVE is-equal
    against an iota constant), bt[e,126]=valid, bt[e,127]=dist, and
    c_aug = [x_block @ W2 ; b_msg ; w3]. One matmul applies the col-side
    message term, the bias, and the dist*w3 term at once.
      msg = relu(pre)                   (ACT)
      aggr_block += bt[:,0:126]^T @ msg (PE, PSUM accumulation over tiles)
    Stationary matmul operands are bf16 (2x faster LDWEIGHTS).
  * Node update per block is a few 128x128 matmuls against the kept x^T.
"""
import numpy as np
import ml_dtypes
import os
BF16 = bool(int(os.environ.get("K_BF16", "1")))

N = 50000
E = 800000
C = 128
H = 128
NCORES = 8
BLK = 126                    # max nodes per block
TB = 16                      # tiles per block (uniform)
ECAP = TB * 128              # max edges per block
NODES_PER_CORE = 6300        # fixed contiguous node range per core


def _build_and_run(in_maps, NB):
    import concourse.bacc as bacc
    import concourse.tile as tile
    from concourse import bass, mybir
    from concourse.bass_utils import run_bass_kernel_spmd

    f32 = mybir.dt.float32
    bf16 = mybir.dt.bfloat16 if BF16 else mybir.dt.float32
    i32 = mybir.dt.int32
    P = 128
    T = TB

    nc = bacc.Bacc("TRN2")
    nc.cache_partition_id()

    node_embed = nc.dram_tensor("node_embed", [N, C], f32, kind="ExternalInput")
    x_blocks = nc.dram_tensor("x_blocks", [NB, P, C], f32, kind="ExternalInput")
    gidx = nc.dram_tensor("gidx", [NB, P, T], i32, kind="ExternalInput")
    colmod = nc.dram_tensor("colmod", [NB, P, T], f32, kind="ExternalInput")
    tail = nc.dram_tensor("tail", [NB, P, 2 * T], f32, kind="ExternalInput")
    cmrows = nc.dram_tensor("cmrows", [NB, T, P], f32, kind="ExternalInput")
    tailrows = nc.dram_tensor("tailrows", [NB, T, 2, P], bf16, kind="ExternalInput")
    iotap_d = nc.dram_tensor("iotap", [P, P], f32, kind="ExternalInput")
    W1 = nc.dram_tensor("W1", [C, H], bf16, kind="ExternalInput")
    W2 = nc.dram_tensor("W2", [C, H], f32, kind="ExternalInput")
    bmsg_w3 = nc.dram_tensor("bmsg_w3", [2, H], bf16, kind="ExternalInput")
    Wu1 = nc.dram_tensor("Wu1", [C, H], f32, kind="ExternalInput")
    Wu2 = nc.dram_tensor("Wu2", [H, H], f32, kind="ExternalInput")
    Wres = nc.dram_tensor("Wres", [C, H], f32, kind="ExternalInput")
    bupd = nc.dram_tensor("bupd", [H, 1], f32, kind="ExternalInput")
    iota = nc.dram_tensor("iota", [P, P], f32, kind="ExternalInput")
    ident = nc.dram_tensor("ident", [P, P], f32, kind="ExternalInput")
    identb = nc.dram_tensor("identb", [P, P], bf16, kind="ExternalInput")
    out_d = nc.dram_tensor("out", [NB * BLK, H], f32, kind="ExternalOutput")

    RELU = mybir.ActivationFunctionType.Relu
    EQ = mybir.AluOpType.is_equal

    with tile.TileContext(nc) as tc:
        with tc.tile_pool(name="const", bufs=1) as cp, \
             tc.tile_pool(name="persist", bufs=1) as pp, \
             tc.tile_pool(name="work", bufs=8) as wp, \
             tc.tile_pool(name="xg", bufs=16) as gp, \
             tc.tile_pool(name="psum3", bufs=2, space="PSUM") as ps, \
             tc.tile_pool(name="psum2", bufs=2, space="PSUM") as ps2:

            def load_const(t, name):
                tl = cp.tile(list(t.shape), t.dtype, tag=name)
                nc.sync.dma_start(out=tl[:], in_=t[:])
                return tl

            w1 = load_const(W1, "w1")
            w2 = load_const(W2, "w2")
            wu1 = load_const(Wu1, "wu1")
            wu2 = load_const(Wu2, "wu2")
            wres = load_const(Wres, "wres")
            bu = load_const(bupd, "bu")
            io_t = load_const(iota, "iota")
            io_p = load_const(iotap_d, "iotap")
            idt = load_const(ident, "ident")
            idtb = load_const(identb, "identb")

            # ---------- phase C: per-block x^T and c_aug ----------
            xT = []
            caug = []
            for b in range(NB):
                xb = wp.tile([P, C], f32, tag="xb")
                nc.sync.dma_start(out=xb[:], in_=x_blocks[b])
                ptx = ps.tile([P, P], f32, space="PSUM", tag="ptx")
                nc.tensor.transpose(out=ptx[:], in_=xb[:], identity=idt[:])
                xt = pp.tile([C, P], f32, tag=f"xT{b}")
                nc.scalar.copy(out=xt[:], in_=ptx[:])
                pc = ps.tile([P, H], f32, space="PSUM", tag="pre")
                nc.tensor.matmul(out=pc[:], lhsT=xt[:], rhs=w2[:], start=True, stop=True)
                ca = pp.tile([P, H], bf16, tag=f"caug{b}")
                nc.vector.tensor_copy(out=ca[0:BLK, :], in_=pc[0:BLK, :])
                nc.sync.dma_start(out=ca[BLK:P, :], in_=bmsg_w3[:])
                xT.append(xt)
                caug.append(ca)

            # ---------- phase E: edges; phase U: node update ----------
            for b in range(NB):
                gix = wp.tile([P, T], i32, tag="gix")
                nc.sync.dma_start(out=gix[:], in_=gidx[b])
                cmod = wp.tile([P, T], f32, tag="cmod")
                nc.sync.dma_start(out=cmod[:], in_=colmod[b])
                tl = wp.tile([P, 2 * T], f32, tag="tail")
                nc.sync.dma_start(out=tl[:], in_=tail[b])

                pagg = ps2.tile([P, H], f32, space="PSUM", tag="agg")
                for t in range(T):
                    xg = gp.tile([P, C], f32, tag="xg")
                    nc.gpsimd.indirect_dma_start(
                        out=xg[:], out_offset=None, in_=node_embed[:],
                        in_offset=bass.IndirectOffsetOnAxis(ap=gix[:, t:t + 1], axis=0))
                    ptx = ps.tile([P, P], f32, space="PSUM", tag="ptx")
                    nc.tensor.transpose(out=ptx[:], in_=xg[:], identity=idt[:])
                    xts = wp.tile([C, P], bf16, tag="xts")
                    nc.vector.tensor_copy(out=xts[:], in_=ptx[:])

                    bt = gp.tile([P, P], bf16, tag="bt")
                    nc.vector.tensor_scalar(bt[:], io_t[:], cmod[:, t:t + 1], None, EQ)
                    nc.vector.tensor_copy(out=bt[:, BLK:P], in_=tl[:, 2 * t:2 * t + 2])
                    pbt = ps2.tile([P, P], bf16, space="PSUM", tag="pbt")
                    nc.tensor.transpose(out=pbt[:], in_=bt[:], identity=idtb[:])
                    btT = wp.tile([P, P], bf16, tag="btT")
                    nc.vector.tensor_copy(out=btT[:], in_=pbt[:])

                    ppre = ps.tile([P, H], f32, space="PSUM", tag="pre")
                    nc.tensor.matmul(out=ppre[:], lhsT=xts[:], rhs=w1[:], start=True, stop=False)
                    nc.tensor.matmul(out=ppre[:], lhsT=btT[:], rhs=caug[b][:], start=False, stop=True)
                    msg = wp.tile([P, H], bf16, tag="msg")
                    nc.scalar.activation(out=msg[:], in_=ppre[:], func=RELU)
                    nc.tensor.matmul(out=pagg[0:BLK, :], lhsT=bt[:, 0:BLK], rhs=msg[:],
                                     start=(t == 0), stop=(t == T - 1))

                # ----- node update for block b -----
                aggs = wp.tile([P, H], f32, tag="aggs")
                nc.vector.memset(aggs[:], 0)
                nc.vector.tensor_copy(out=aggs[0:BLK, :], in_=pagg[0:BLK, :])
                pat = ps.tile([P, P], f32, space="PSUM", tag="ptx")
                nc.tensor.transpose(out=pat[:], in_=aggs[:], identity=idt[:])
                aggT = wp.tile([H, P], f32, tag="aggT")
                nc.vector.tensor_copy(out=aggT[:], in_=pat[:])

                pupd = ps.tile([H, P], f32, space="PSUM", tag="pre")
                nc.tensor.matmul(out=pupd[:], lhsT=wu1[:], rhs=xT[b][:], start=True, stop=False)
                nc.tensor.matmul(out=pupd[:], lhsT=wu2[:], rhs=aggT[:], start=False, stop=True)
                rel = wp.tile([H, P], f32, tag="rel")
                nc.scalar.activation(out=rel[:], in_=pupd[:], func=RELU, bias=bu[:])

                pout = ps.tile([H, P], f32, space="PSUM", tag="ptx")
                nc.tensor.matmul(out=pout[:], lhsT=wres[:], rhs=xT[b][:], start=True, stop=True)
                outT = wp.tile([H, P], f32, tag="outT")
                nc.vector.tensor_tensor(out=outT[:], in0=pout[:], in1=rel[:],
                                        op=mybir.AluOpType.add)
                pfin = ps.tile([P, H], f32, space="PSUM", tag="ptx")
                nc.tensor.transpose(out=pfin[:], in_=outT[:], identity=idt[:])
                outs = wp.tile([P, H], f32, tag="outs")
                nc.scalar.copy(out=outs[:], in_=pfin[:])
                nc.sync.dma_start(out=out_d[b * BLK:(b + 1) * BLK, :], in_=outs[0:BLK, :])

    nc.finalize()
    res = run_bass_kernel_spmd(nc, in_maps, core_ids=list(range(NCORES)),
                               trace=bool(int(__import__("os").environ.get("K_TRACE", "0"))))
    return res


def kernel(node_embed, edge_dist, edge_index, W_res, W_msg, b_msg, W_upd, b_upd):
    node_embed = np.asarray(node_embed, dtype=np.float32)
    edge_dist = np.asarray(edge_dist, dtype=np.float32).reshape(-1)
    row = np.asarray(edge_index[0], dtype=np.int64).astype(np.int32)
    col = np.asarray(edge_index[1], dtype=np.int64).astype(np.int32)
    W_res = np.asarray(W_res, dtype=np.float32)
    W_msg = np.asarray(W_msg, dtype=np.float32)
    b_msg = np.asarray(b_msg, dtype=np.float32)
    W_upd = np.asarray(W_upd, dtype=np.float32)
    b_upd = np.asarray(b_upd, dtype=np.float32)

    order = np.argsort(col, kind="stable")
    scol = col[order]
    srow = row[order]
    sdist = edge_dist[order]

    # per-core greedy blocks: <=BLK nodes, <=ECAP edges
    core_blocks = []   # per core: list of (node_start, node_end, e0, e1)
    for core in range(NCORES):
        n0 = core * NODES_PER_CORE
        n1 = min(n0 + NODES_PER_CORE, N)
        blocks = []
        v = n0
        while v < n1:
            vmax = min(v + BLK, n1)
            e0 = np.searchsorted(scol, v)
            emax = np.searchsorted(scol, vmax)
            if emax - e0 <= ECAP:
                vend = vmax
                e1 = emax
            else:
                # find largest vend with edge count <= ECAP
                e1 = e0 + ECAP
                vend = int(scol[e1 - 1])  # last fully-included node candidate
                # all edges of node vend must fit; back off to node boundary
                e1 = np.searchsorted(scol, vend)
                vend = max(vend, v + 1)
                e1 = np.searchsorted(scol, vend)
            blocks.append((v, vend, int(e0), int(e1)))
            v = vend
        core_blocks.append(blocks)

    NB = max(len(b) for b in core_blocks)
    P = 128
    T = TB
    gidx = np.zeros((NCORES, NB, P, T), np.int32)
    colm = np.full((NCORES, NB, P, T), -1.0, np.float32)
    tailh = np.zeros((NCORES, NB, P, 2 * T), np.float32)
    x_blocks = np.zeros((NCORES, NB, P, C), np.float32)

    for core in range(NCORES):
        for b, (v0, v1, e0, e1) in enumerate(core_blocks[core]):
            cnt = e1 - e0
            if cnt:
                idx = np.arange(cnt)
                tt, pp_ = idx // 128, idx % 128
                gidx[core, b, pp_, tt] = srow[e0:e1]
                colm[core, b, pp_, tt] = (scol[e0:e1] - v0).astype(np.float32)
                tailh[core, b, pp_, 2 * tt] = 1.0
                tailh[core, b, pp_, 2 * tt + 1] = sdist[e0:e1]
            x_blocks[core, b, 0:v1 - v0, :] = node_embed[v0:v1]

    iota = np.tile(np.arange(P, dtype=np.float32), (P, 1))
    iota[:, BLK:] = -5.0
    iotap = np.repeat(np.arange(P, dtype=np.float32)[:, None], P, axis=1)
    iotap[BLK:, :] = -6.0
    cmrows = np.ascontiguousarray(colm.transpose(0, 1, 3, 2))
    tailrows = np.ascontiguousarray(
        tailh.reshape(NCORES, NB, P, T, 2).transpose(0, 1, 3, 4, 2)
    ).astype(ml_dtypes.bfloat16 if BF16 else np.float32)
    consts = {
        "W1": W_msg[0:C].astype(ml_dtypes.bfloat16 if BF16 else np.float32),
        "W2": W_msg[C:2 * C],
        "bmsg_w3": np.stack([b_msg, W_msg[2 * C]]).astype(ml_dtypes.bfloat16 if BF16 else np.float32),
        "Wu1": W_upd[0:C], "Wu2": W_upd[C:C + H],
        "Wres": W_res, "bupd": b_upd.reshape(H, 1),
        "iota": iota, "iotap": iotap, "ident": np.eye(P, dtype=np.float32),
        "identb": np.eye(P).astype(ml_dtypes.bfloat16 if BF16 else np.float32),
    }
    in_maps = []
    for core in range(NCORES):
        m = {"node_embed": node_embed, "x_blocks": x_blocks[core],
             "gidx": gidx[core], "colmod": colm[core], "tail": tailh[core],
             "cmrows": cmrows[core], "tailrows": tailrows[core]}
        m.update(consts)
        in_maps.append(m)

    res = _build_and_run(in_maps, NB)
    kernel._last_result = res

    out = np.empty((N, H), np.float32)
    for core in range(NCORES):
        o = res.results[core]["out"]
        for b, (v0, v1, _, _) in enumerate(core_blocks[core]):
            out[v0:v1] = o[b * BLK:b * BLK + (v1 - v0)]
    return out
